# revision 6
# baseline (speedup 1.0000x reference)
"""Trainium2 Bass kernel for EventPropLinear forward (LIF spiking layer), v4.

out[b,o,t]: spikes of an LIF layer driven by J = W @ x through
    I[t] = a_s*I[t-1] + J[t];  V[t] = a_m*V[t-1] + b_m*I[t-1]
    spike = V > 1 -> V resets to 0.

Design (8 cores, data-parallel over batch, 16 samples/core):
  - x ships as fp8e4 (0/1 exact), packed host-side into the SBUF layout
    [ki=112, b, k, t] in two chunk-groups so the first GEMM chunk's data
    lands early; 2 input DMAs, 112 contiguous descriptors each.
  - GEMM on TensorE: bf16 2-split weights (W = W1+W2, b_m folded) x fp8 spikes
    -> fp32 PSUM, chunked along time for pipelining.
  - I-recurrence: ACT copies PSUM J -> SBUF time-major bi[128, t, s]
    (s = b*4+c). DVE runs 4 batched masked-restart scans per chunk over the
    series-major VIEW of bi: data0 is an a_s tile with 0 at each series'
    t=0 so the scan state restarts per series; per-series carries are
    pre-added into column 0 with one STT. This amortizes the per-op scan
    cost over 16 series (64 scans/chunk -> 4).
  - V-recurrence: serial 2-STT-per-step DVE loop on contiguous [128,64] cols,
    emitted at the start of the NEXT chunk so it overlaps that chunk's GEMM.
  - Spike extract: Pool is_gt -> u8 staging [128, 500*64]; staged output
    DMAs (128 descriptors each); host reorders to (B, 512, 500) fp32.
"""
import numpy as np
import ml_dtypes
import concourse.bass as bass
import concourse.bacc as bacc
import concourse.mybir as mybir
import concourse.tile as tile
from concourse.bass_utils import run_bass_kernel_spmd
from contextlib import ExitStack

B, IN_DIM, OUT_DIM, STEPS = 128, 784, 512, 500
NCORES = 8
BL = B // NCORES            # 16 samples per core
KC, NK = 112, 7             # contraction chunking: 784 = 7*112
NC_ = OUT_DIM // 128        # 4 o-chunks
NSER = BL * NC_             # 64 series (b, o-chunk)
SG = 8                      # series per batched scan (8 scans per chunk)
TX = 512                    # padded x time (only cols 0..497 used)
CHUNKS_J = [32, 120, 120, 120, 106]  # sum = 498 (j runs 2..500)
TG1 = 152                   # x chunk-group 1: t in [0, 152) = chunks 1-2
JDRAIN = (274, 394)         # staged output drains at these j boundaries
a_m = 1.0 - 0.1 / 20.0      # 0.995
b_m = 0.1 / 20.0            # 0.005
a_s = 1.0 - 0.1 / 5.0       # 0.98
f32, bf16 = mybir.dt.float32, mybir.dt.bfloat16
fp8, u8 = mybir.dt.float8e4, mybir.dt.uint8

_cache = {}


def _build():
    nc = bacc.Bacc()
    wpk = nc.declare_dram_parameter("wpk", [KC, NK * 2 * NC_ * 128], bf16, isOutput=False)
    xp = nc.declare_dram_parameter("xp", [KC, BL * NK * TX], fp8, isOutput=False)
    outp = nc.declare_dram_parameter("outp", [128, STEPS * NSER], u8, isOutput=True)

    MULT, ADD = mybir.AluOpType.mult, mybir.AluOpType.add
    ISLE, ISGT = mybir.AluOpType.is_le, mybir.AluOpType.is_gt
    COPYF = mybir.ActivationFunctionType.Copy
    CHM = max(CHUNKS_J)
    NG1 = BL * NK * TG1                 # fp8 elements per partition in group 1

    with tile.TileContext(nc) as tc, ExitStack() as ctx:
        sb = ctx.enter_context(tc.tile_pool(name="sb", bufs=1))
        bipool = ctx.enter_context(tc.tile_pool(name="bi", bufs=3))
        pspool = ctx.enter_context(tc.tile_pool(name="ps", bufs=8, space="PSUM"))

        wt = sb.tile([KC, NK * 2 * NC_ * 128], bf16, tag="wt")
        nc.scalar.dma_start(wt[:], wpk[:, :])   # ACT queue, parallel with x g1
        xs = sb.tile([KC, BL * NK * TX], fp8, tag="xs")
        nc.sync.dma_start(xs[:, :NG1], xp[:, :NG1])
        nc.sync.dma_start(xs[:, NG1:], xp[:, NG1:])
        xsv1 = xs[:, :NG1].rearrange("p (b k t) -> p b k t", b=BL, k=NK)
        xsv2 = xs[:, NG1:].rearrange("p (b k t) -> p b k t", b=BL, k=NK)

        # masked-restart scan multipliers (one per distinct chunk size):
        # a_s everywhere, 0 at each series' t=0 so the state restarts
        msks = {}
        for CH_ in sorted(set(CHUNKS_J)):
            m = sb.tile([128, SG * CH_], f32, tag=f"msk{CH_}")
            nc.vector.memset(m[:], a_s)
            nc.vector.memset(m[:].rearrange("p (s t) -> p s t", t=CH_)[:, :, 0:1], 0.0)
            msks[CH_] = m
        stg = sb.tile([128, STEPS * NSER], u8, tag="stg")
        nc.gpsimd.memset(stg[:, 0:2 * NSER], 0)   # out cols j=0,1 are zero
        vr = sb.tile([128, NSER], f32, tag="vr")
        nc.vector.memset(vr[:], 0.0)
        carry = sb.tile([128, NSER], f32, tag="carry")
        nc.vector.memset(carry[:], 0.0)

        def v_steps(biv_, CH_):
            # V steps of a whole chunk: col j holds bI[j-2]; after the
            # update it holds u_{j-1}; vr keeps the reset state.
            for t in range(CH_):
                col = biv_[:, t, :]
                nc.vector.scalar_tensor_tensor(col, vr[:], a_m, col, MULT, ADD)
                nc.vector.scalar_tensor_tensor(vr[:], col, 1.0, col, ISLE, MULT)

        stgv = stg[:].rearrange("p (t s) -> p t s", s=NSER)

        def extract(biv_, bi_, j0_, CH_):
            # spikes for cols [j0_, j0_+CH_) -> u8 staging (per-series: src is
            # series-major contiguous, dst is time-major strided)
            bsm = bi_[:].rearrange("p (s t) -> p s t", s=NSER)
            for s_ in range(NSER):
                nc.gpsimd.tensor_scalar(stgv[:, j0_:j0_ + CH_, s_],
                                        bsm[:, s_, :], 1.0, None, ISGT)
            j1 = j0_ + CH_
            if j1 == JDRAIN[0]:
                nc.sync.dma_start(outp[:, :j1 * NSER], stg[:, :j1 * NSER])
            elif j1 == JDRAIN[1]:
                nc.sync.dma_start(outp[:, JDRAIN[0] * NSER:j1 * NSER],
                                  stg[:, JDRAIN[0] * NSER:j1 * NSER])

        prev = None                     # (biv, bi, j0, CH) of previous chunk
        j0 = 2
        for ci, CH in enumerate(CHUNKS_J):
            t0 = j0 - 2                 # x/J column offset for this chunk
            bi = bipool.tile([128, CH * NSER], f32, tag="bi")
            biv = bi[:].rearrange("p (s t) -> p t s", s=NSER)   # strided t-cols
            biv_sm = bi[:].rearrange("p (s t) -> p s t", s=NSER)
            for b in range(BL):
                # previous chunk's V-slice: DVE chews the serial chain while
                # PE produces this chunk's J
                if prev is not None:
                    pCH = prev[3]
                    for t in range(pCH * b // BL, pCH * (b + 1) // BL):
                        col = prev[0][:, t, :]
                        nc.vector.scalar_tensor_tensor(col, vr[:], a_m, col, MULT, ADD)
                        nc.vector.scalar_tensor_tensor(vr[:], col, 1.0, col, ISLE, MULT)
                for c in range(NC_):
                    s = b * NC_ + c
                    p = pspool.tile([128, CH], f32, tag="ps")
                    for k in range(NK):
                        for sp in range(2):
                            w0 = ((k * 2 + sp) * NC_ + c) * 128
                            xv = (xsv1[:, b, k, t0:t0 + CH] if t0 + CH <= TG1
                                  else xsv2[:, b, k, t0 - TG1:t0 - TG1 + CH])
                            nc.tensor.matmul(p[:], wt[:, w0:w0 + 128], xv,
                                             start=(k == 0 and sp == 0),
                                             stop=(k == NK - 1 and sp == 1))
                    nc.scalar.activation(biv_sm[:, s, :], p[:], COPYF)
                # after each 2-sample group (8 series), fix up carries into
                # col 0 and run one batched masked-restart scan over 8 series
                if b % 2 == 1:
                    g0 = (b - 1) * NC_
                    nc.vector.scalar_tensor_tensor(
                        biv[:, 0, g0:g0 + SG], carry[:, g0:g0 + SG], a_s,
                        biv[:, 0, g0:g0 + SG], MULT, ADD)
                    seg = bi[:, g0 * CH:(g0 + SG) * CH]
                    nc.vector.tensor_tensor_scan(
                        seg, msks[CH][:], seg, 0.0, MULT, ADD)
            # chunk-boundary bI (scan state) for the next chunk's init
            if ci + 1 < len(CHUNKS_J):
                nc.gpsimd.tensor_copy(carry[:], biv[:, CH - 1, :])
            if prev is not None:
                extract(*prev)
            prev = (biv, bi, j0, CH)
            j0 += CH
        # tail: last chunk's V + extract + final drains
        v_steps(prev[0], prev[3])
        extract(*prev)
        nc.sync.dma_start(outp[:, JDRAIN[1] * NSER:], stg[:, JDRAIN[1] * NSER:])
    nc.finalize()
    return nc


def _prep_weights(weight):
    ws = (b_m * weight.astype(np.float64)).astype(np.float32)
    w1 = ws.astype(ml_dtypes.bfloat16)
    w2 = (ws - w1.astype(np.float32)).astype(ml_dtypes.bfloat16)
    wpk = np.zeros((KC, NK, 2, NC_, 128), ml_dtypes.bfloat16)
    for k in range(NK):
        for c in range(NC_):
            wpk[:, k, 0, c, :] = w1[c * 128:(c + 1) * 128, k * KC:(k + 1) * KC].T
            wpk[:, k, 1, c, :] = w2[c * 128:(c + 1) * 128, k * KC:(k + 1) * KC].T
    return np.ascontiguousarray(wpk.reshape(KC, -1))


def _prep_x(xc):
    """xc: (BL, 784, 500) float 0/1 -> fp8e4 [112, BL*7*512], two chunk-groups
    (t < TG1 first, then t >= TG1) each contiguous per partition."""
    xpk = np.zeros((KC, BL, NK, TX), ml_dtypes.float8_e4m3fn)
    xr = xc[:, :, :STEPS - 2].reshape(BL, NK, KC, STEPS - 2).transpose(2, 0, 1, 3)
    xpk[:, :, :, :STEPS - 2] = xr.astype(ml_dtypes.float8_e4m3fn)
    g1 = xpk[:, :, :, :TG1].reshape(KC, -1)
    g2 = xpk[:, :, :, TG1:].reshape(KC, -1)
    return np.ascontiguousarray(np.concatenate([g1, g2], axis=1))


def _unpack_out(stg):
    """stg: [128, 500*64] u8 -> (BL, 512, 500) float32."""
    s = stg.reshape(128, STEPS, BL, NC_).transpose(2, 3, 0, 1)
    return s.reshape(BL, OUT_DIM, STEPS).astype(np.float32)


def kernel(x, weight):
    if "nc" not in _cache:
        _cache["nc"] = _build()
    nc = _cache["nc"]
    wpk = _prep_weights(weight)
    in_maps = [{"wpk": wpk, "xp": _prep_x(x[i * BL:(i + 1) * BL])}
               for i in range(NCORES)]
    res = run_bass_kernel_spmd(nc, in_maps, list(range(NCORES)))
    return np.concatenate([_unpack_out(res.results[i]["outp"])
                           for i in range(NCORES)], axis=0)
